# revision 41
# baseline (speedup 1.0000x reference)
"""Evo2 attention (B=2, S=2048, HID=2048, NH=16, HD=128) on 8 trn2 NeuronCores.

Sharding: core c handles batch b=c//4 and heads 4*(c%4)..4*(c%4)+3.
Megatron-style: q/k/v projections column-parallel, o_proj row-parallel with the
4-way partial sum done on host during unshard.

Per-core kernel layout (everything transposed so no on-chip transposes needed):
  hsT [hid, tok] -> qT,kT [hd, tok] (RoPE fused into PSUM eviction, rotate-half
  basis obtained by de-interleaving W rows on host), v [tok, hd].
  scoresT[k, q] = kT_blk vs qT matmul; softmax over k (= partitions) with a
  fixed shift instead of a max; denominators via ones-vector PE reduction and
  a K=1 matmul broadcast; PV gives attnT [hd, q]; o_projT partial [o, q].

Optimizations vs the fp32r baseline (429us -> ~339us):
  - all matmul operands bf16 (PSUM accumulation stays fp32): halves DMA+SBUF,
    enables FWL weight loads; output partials bf16, summed fp32 on host
  - phase A: one hs residency per 512-token chunk feeds q, k AND v passes
    (hs loaded once, not twice); dblk-outer groups so rope evictions stagger;
    weights DMA'd in 2-kc pieces interleaved with hs tiles (short head)
  - PE warmup matmuls during the initial DMA fill (HAM un-throttle) and an
    early dummy exp (ACT table-set preload off the critical path)
  - causal diagonal blocks column-trimmed: fully-masked leading q-columns of
    each [128k x 512q] tile are never computed (score/exp/den/PV all skip)
  - reciprocal -> single-pass reciprocal_approx_fast (~5x faster DVE op)
  - denominator reductions: off-diagonal probs tiles summed pairwise on DVE
    (bf16 2x) so half those N=512 PE streams disappear
  - dependency-granularity fixes: per-(head,chunk) kt tiles, perm-scoped mask
    pool with DMAs issued after the weight pieces (Tile deps are per-tile and
    pool SBUF ranges alias phase A's, which otherwise serializes phase B)
  - o-projection of the previous q-chunk interleaved INTO the score/PV loop
    at block granularity as PE filler for the exp-chain waits
"""
import os
import sys
import numpy as np

for _p in ("/opt/trn_rl_repo",):
    if os.path.isdir(_p) and _p not in sys.path:
        sys.path.insert(0, _p)

import ml_dtypes

BF16 = ml_dtypes.bfloat16
B, S, HID, NH = 2, 2048, 2048, 16
HD = HID // NH            # 128
HPC = 4                   # heads per core
NCORES = 8
BASE = 10000.0
SCALE = 1.0 / np.sqrt(HD).astype(np.float32)
SHIFT = 25.0              # fixed softmax shift (replaces per-row max)
NEG_INF_THRESH = -1e8

_PROGRAM_CACHE = {}


def _rope_tables():
    """cos/±sin tables [HD, S] in the de-interleaved (rotate-half) basis.

    Reference pairs dims (2m, 2m+1) with angle theta_m(s) = s * inv_freq[f(m)],
    f(m) = 2m for m<32 else 2m-64 (from emb[:, ::2] of concat([freqs, freqs])).
    After de-interleave perm [0,2,..126,1,3,..127]: new dim m<64 is old 2m,
    new dim 64+m is old 2m+1.
      out[m]    = x[m] cos_m - x[64+m] sin_m
      out[64+m] = x[m] sin_m + x[64+m] cos_m
    """
    inv_freq = BASE ** (-np.arange(0, HD, 2, dtype=np.float64) / HD)  # [64]
    m = np.arange(64)
    fmap = np.where(m < 32, 2 * m, 2 * m - 64)
    t = np.arange(S, dtype=np.float64)
    theta = t[None, :] * inv_freq[fmap][:, None]          # [64, S]
    cos = np.cos(theta)
    sin = np.sin(theta)
    cosT = np.concatenate([cos, cos], axis=0).astype(np.float32)      # [128, S]
    # row d holds the factor applied to SOURCE half d (dest = other half):
    # src lo -> dst hi uses +sin; src hi -> dst lo uses -sin
    ssinT = np.concatenate([sin, -sin], axis=0).astype(np.float32)    # [128, S]
    return cosT, ssinT


def _mask_plan(mask2d):
    """Classify [128k x 512q] blocks of mask^T. Returns (plan, tiles).

    plan[qc] = list of (kb, mask_tile_idx_or_None, col_off); fully-masked
    blocks skipped; col_off = count of leading fully-masked q-columns (those
    columns are skipped entirely in score/exp/den/PV).
    tiles: deduped f32 [128, 512] mask^T blocks prescaled by sqrt(HD).
    """
    maskT = np.ascontiguousarray(mask2d.T)  # [k, q]
    plan = []
    tiles = []
    seen = {}
    for qc in range(S // 512):
        row = []
        for kb in range(S // 128):
            sub = maskT[kb * 128:(kb + 1) * 128, qc * 512:(qc + 1) * 512]
            if (sub <= NEG_INF_THRESH).all():
                continue
            colmasked = (sub <= NEG_INF_THRESH).all(axis=0)  # [512]
            off = 0
            while off < 512 and colmasked[off]:
                off += 1
            if (sub[:, off:] == 0.0).all():
                row.append((kb, None, off))
                continue
            pre = np.ascontiguousarray(sub * np.float32(np.sqrt(HD)))
            key = pre.tobytes()
            idx = seen.get(key)
            if idx is None:
                idx = len(tiles)
                seen[key] = idx
                tiles.append(pre)
            row.append((kb, idx, off))
        plan.append(row)
    return plan, tiles


def _build_program(plan, nm, preload_masks):
    import contextlib
    import concourse.bacc as bacc
    import concourse.tile as tile
    from concourse import mybir

    f32 = mybir.dt.float32
    f32r = mybir.dt.float32r
    bf16 = mybir.dt.bfloat16
    nc = bacc.Bacc(None, target_bir_lowering=False)

    # host-pre-tiled inputs: last axis group per partition is contiguous
    hs_d = nc.dram_tensor("hs_t", [4, 128, 16, 512], bf16, kind="ExternalInput")
    wq_d = nc.dram_tensor("wq_t", [128, 16, 4, 128], bf16, kind="ExternalInput")
    wk_d = nc.dram_tensor("wk_t", [128, 16, 4, 128], bf16, kind="ExternalInput")
    wv_d = nc.dram_tensor("wv_t", [128, 16, 512], bf16, kind="ExternalInput")
    wo_d = nc.dram_tensor("wo_t", [128, 4, 16, 128], bf16, kind="ExternalInput")
    cos_d = nc.dram_tensor("cosT", [128, S], bf16, kind="ExternalInput")
    sin_d = nc.dram_tensor("ssinT", [128, S], bf16, kind="ExternalInput")
    if nm:
        mask_d = nc.dram_tensor("maskt", [nm, 128, 512], bf16, kind="ExternalInput")
    o_d = nc.dram_tensor("oT_t", [4, 16, 128, 512], bf16, kind="ExternalOutput")

    Exp = mybir.ActivationFunctionType.Exp

    with tile.TileContext(nc) as tc:
        with contextlib.ExitStack() as perm:
            kt_pool = perm.enter_context(tc.tile_pool(name="kt", bufs=16))
            qa_pool = perm.enter_context(tc.tile_pool(name="qa", bufs=17))
            cst = perm.enter_context(tc.tile_pool(name="cst", bufs=1))
            tmp_pool = perm.enter_context(tc.tile_pool(name="tmp", bufs=2))
            wo_pool = perm.enter_context(tc.tile_pool(name="wo", bufs=1))

            # ---- PE warmup: ~3.5us of dummy matmuls while DMAs fill ------
            wup_st = cst.tile([128, 512], f32, tag="wu0")
            nc.vector.memset(wup_st, 0.0)
            wup = cst.tile([128, 512], bf16, tag="wu1")
            nc.vector.tensor_copy(out=wup, in_=wup_st)

            onesq_st = cst.tile([128, 128], f32, tag="o3")
            nc.vector.memset(onesq_st, 1.0)
            onesq_r = cst.tile([128, 128], bf16, tag="o4")
            nc.vector.tensor_copy(out=onesq_r, in_=onesq_st)
            ones_r1 = cst.tile([128, 128], f32r, tag="o5")
            nc.vector.tensor_copy(out=ones_r1[0:1, :], in_=onesq_st[0:1, :])
            shiftb = cst.tile([128, 1], f32, tag="sh")
            nc.vector.memset(shiftb, -SHIFT)
            # preload the exp ACT table set now; otherwise the first phase-B
            # exp pays the ~2.7us ACT_TABLE_LOAD + DRAIN on the critical path
            expwarm = cst.tile([128, 1], f32, tag="ew")
            nc.scalar.activation(expwarm[:], shiftb[:], Exp)

            # per-(head, chunk) kt tiles: tile-granular dependency tracking
            # means one big [128,S] tile would gate phase B's first score
            # matmul on the LAST chunk's rope writes
            kt = [[kt_pool.tile([128, 512], bf16, tag="kt", name=f"kt{i}_{c}")
                   for c in range(4)] for i in range(HPC)]

            # mask tiles live in a perm-scoped pool: allocated inside phase
            # B's stack their SBUF range overlaps phase A's pools, which makes
            # the DMA wait for ALL phase-A matmuls and puts ~5us of mask
            # latency on the first exp chain. The DMAs themselves are issued
            # later (after the weight pieces) so they don't delay the head.
            mask_sb = None
            if nm and preload_masks:
                mkp = perm.enter_context(tc.tile_pool(name="mk", bufs=nm))
                mask_sb = [mkp.tile([128, 512], bf16, tag="mk", name=f"mk{i}")
                           for i in range(nm)]
            v_tiles = [None] * 16
            q_tiles = {}
            attn_tiles = {}

            def rope_evict(ps, dst, cos_sb, sin_sb):
                nc.vector.tensor_mul(dst[0:64, :], ps[64:128, :], sin_sb[64:128, :])
                nc.vector.tensor_mul(dst[64:128, :], ps[0:64, :], sin_sb[0:64, :])
                t = tmp_pool.tile([128, 512], f32, tag="ropetmp")
                nc.vector.tensor_mul(t[:], ps[:], cos_sb[:, :])
                nc.vector.tensor_add(dst[:, :], dst[:, :], t[:])

            # ---- Phase A: projections. One hs residency per 512-token
            # chunk feeds the q, k AND v passes (hs loaded once). kc-outer
            # loop order: the first matmuls gate only on the first kc slice
            # of weights + first hs tile.
            with contextlib.ExitStack() as actx:
                hs_pool = actx.enter_context(tc.tile_pool(name="hs", bufs=8))
                tabp = actx.enter_context(tc.tile_pool(name="tab", bufs=4))
                w_pool = actx.enter_context(tc.tile_pool(name="w", bufs=1))
                # 4+3 = 7 banks: one PSUM bank stays free so phase B's first
                # score matmul isn't gated on phase A's full eviction chain
                psA = actx.enter_context(tc.tile_pool(name="psA", bufs=4, space="PSUM"))
                psAv = actx.enter_context(tc.tile_pool(name="psAv", bufs=3, space="PSUM"))
                v_pool = perm.enter_context(tc.tile_pool(name="v", bufs=16, side="right"))

                def load_hs(c):
                    qts = []
                    for hf in range(4):
                        ht = hs_pool.tile([128, 4, 512], bf16, tag="hs",
                                          name=f"hs{c}_{hf}")
                        nc.sync.dma_start(
                            out=ht, in_=hs_d[c, :, hf * 4:(hf + 1) * 4, :])
                        qts.append(ht)
                    return qts

                # warmup matmuls fill the PE while the first DMAs land
                # (~12us of dummy work: HAM un-throttles and PE is hot when
                # the first real operands arrive)
                ps_wu = psA.tile([128, 512], f32, tag="qk", name="pswu")
                for i in range(50):
                    nc.tensor.matmul(ps_wu[:], wup[:, 0:128], wup[:],
                                     start=True, stop=True)

                wq_all = w_pool.tile([128, 16, 4, 128], bf16, tag="wq", name="wqall")
                wk_all = w_pool.tile([128, 16, 4, 128], bf16, tag="wk", name="wkall")
                wv_all = w_pool.tile([128, 16, 512], bf16, tag="wv", name="wvall")
                wo_all = wo_pool.tile([128, 4, 16, 128], bf16, tag="wo", name="woall")
                # interleave hs chunk-0 tiles with wq pieces in consumption
                # order: the first accumulation group reads (ht[kc//4], wq[kc])
                # kc-ascending, so the head is gated on ~0.8MB, not 4MB
                hs_cur = []
                for hf in range(4):
                    ht = hs_pool.tile([128, 4, 512], bf16, tag="hs", name=f"hs0_{hf}")
                    nc.sync.dma_start(out=ht, in_=hs_d[0, :, hf * 4:(hf + 1) * 4, :])
                    hs_cur.append(ht)
                    for p in (2 * hf, 2 * hf + 1):
                        nc.sync.dma_start(out=wq_all[:, p * 2:(p + 1) * 2, :, :],
                                          in_=wq_d[:, p * 2:(p + 1) * 2, :, :])
                for p in range(8):
                    nc.sync.dma_start(out=wk_all[:, p * 2:(p + 1) * 2, :, :],
                                      in_=wk_d[:, p * 2:(p + 1) * 2, :, :])
                for p in range(4):
                    nc.sync.dma_start(out=wv_all[:, p * 4:(p + 1) * 4, :],
                                      in_=wv_d[:, p * 4:(p + 1) * 4, :])
                for p in range(4):
                    nc.sync.dma_start(out=wo_all[:, p, :, :],
                                      in_=wo_d[:, p, :, :])
                if mask_sb is not None:
                    for i in range(nm):
                        nc.sync.dma_start(out=mask_sb[i], in_=mask_d[i, :, :])
                for c in range(4):
                    hs_nxt = load_hs(c + 1) if c < 3 else None
                    cos_sb = tabp.tile([128, 512], bf16, tag="cos")
                    nc.gpsimd.dma_start(out=cos_sb, in_=cos_d[:, c * 512:(c + 1) * 512])
                    sin_sb = tabp.tile([128, 512], bf16, tag="sin")
                    nc.gpsimd.dma_start(out=sin_sb, in_=sin_d[:, c * 512:(c + 1) * 512])
                    # q pass then k pass (dblk-outer: evictions stagger so the
                    # rope DVE work overlaps the next accumulation group)
                    for pass_i, w_all in ((0, wq_all), (1, wk_all)):
                        for dblk in range(4):
                            ps = psA.tile([128, 512], f32, tag="qk",
                                          name=f"ps{pass_i}_{c}_{dblk}")
                            for kc in range(16):
                                nc.tensor.matmul(
                                    ps[:], w_all[:, kc, dblk, :],
                                    hs_cur[kc // 4][:, kc % 4, :],
                                    start=(kc == 0), stop=(kc == 15))
                            if pass_i == 0:
                                q = qa_pool.tile([128, 512], bf16, tag="qa")
                                rope_evict(ps, q, cos_sb, sin_sb)
                                q_tiles[(dblk, c)] = q
                            else:
                                rope_evict(ps[:, :], kt[dblk][c][:, :],
                                           cos_sb, sin_sb)
                    # v pass (vblk-outer, evictions stagger onto ACT)
                    for vblk in range(4):
                        psv = psAv.tile([128, 512], f32, tag="v",
                                        name=f"psv{c}_{vblk}")
                        for kc in range(16):
                            nc.tensor.matmul(
                                psv[:],
                                hs_cur[kc // 4][:, kc % 4, vblk * 128:(vblk + 1) * 128],
                                wv_all[:, kc, :], start=(kc == 0), stop=(kc == 15))
                        vt = v_pool.tile([128, 512], bf16, tag="v", name=f"v{c}_{vblk}")
                        nc.scalar.copy(out=vt[:], in_=psv[:])
                        v_tiles[c * 4 + vblk] = vt
                    hs_cur = hs_nxt

            # ---------------- Phase B + C ------------------------------------
            with contextlib.ExitStack() as bctx:
                probs_pool = bctx.enter_context(tc.tile_pool(name="pr", bufs=12))
                dsum_pool = bctx.enter_context(tc.tile_pool(name="ds", bufs=8))
                smx_pool = bctx.enter_context(tc.tile_pool(name="sm", bufs=6))
                den_pool = bctx.enter_context(tc.tile_pool(name="dn", bufs=4))
                rcb_pool = bctx.enter_context(tc.tile_pool(name="rcb", bufs=3))
                outb_pool = bctx.enter_context(tc.tile_pool(name="ob", bufs=4))
                psB_s = bctx.enter_context(tc.tile_pool(name="psBs", bufs=3, space="PSUM"))
                psB_a = bctx.enter_context(tc.tile_pool(name="psBa", bufs=2, space="PSUM"))
                psB_d = bctx.enter_context(tc.tile_pool(name="psBd", bufs=1, space="PSUM"))
                psC = bctx.enter_context(tc.tile_pool(name="psC", bufs=2, space="PSUM"))
                if nm and not preload_masks:
                    mp = bctx.enter_context(tc.tile_pool(name="mk", bufs=8))

                def emit_tail(state):
                    h, qc, ps_att, den_sb = state
                    rcb = rcb_pool.tile([128, 512], f32, tag="rcb")
                    nc.vector.reciprocal_approx_fast(out=rcb[:], in_=den_sb[:])
                    at = qa_pool.tile([128, 512], bf16, tag="qa")
                    nc.vector.tensor_mul(at[:], ps_att[:], rcb[:])
                    attn_tiles[(h, qc)] = at

                def emit_c_chunk(qc, iblks):
                    for iblk in iblks:
                        ps_o = psC.tile([128, 512], f32, tag="o")
                        for jc in range(4):
                            nc.tensor.matmul(ps_o[:], wo_all[:, jc, iblk, :],
                                             attn_tiles[(jc, qc)][:],
                                             start=(jc == 0), stop=(jc == 3))
                        ob = outb_pool.tile([128, 512], bf16, tag="ob")
                        if iblk % 8 < 4:
                            nc.vector.tensor_copy(out=ob[:], in_=ps_o[:])
                        else:
                            nc.scalar.copy(out=ob[:], in_=ps_o[:])
                        nc.sync.dma_start(out=o_d[qc, iblk, :, :], in_=ob[:])

                tail_state = None
                for qc in range(4):
                    kbs = plan[qc]
                    nkb = len(kbs)
                    # den schedule: off==0 blocks group into quads (two DVE
                    # bf16 adds collapse 4 probs tiles into ONE N=512 PE
                    # reduction stream); leftovers pair; diagonals go single.
                    # Den matmuls are emitted a couple of blocks late so the
                    # DVE adds are off the PE's critical path.
                    off0 = [jj for jj, e in enumerate(kbs) if e[2] == 0]
                    diag = [jj for jj, e in enumerate(kbs) if e[2] != 0]
                    quads, rest = [], list(off0)
                    while len(rest) >= 4:
                        quads.append(tuple(rest[:4]))
                        rest = rest[4:]
                    octs, qrest = [], list(quads)
                    while len(qrest) >= 2:
                        octs.append((qrest[0], qrest[1]))
                        qrest = qrest[2:]
                    quads = qrest
                    pairs = []
                    if len(rest) >= 2:
                        pairs.append((rest[0], rest[1]))
                        rest = rest[2:]
                    singles = rest + diag
                    act_at = {}
                    for q4 in quads:
                        act_at.setdefault(q4[1], []).append(("p1", q4))
                        act_at.setdefault(q4[3], []).append(("p3", q4))
                    for oo in octs:
                        q4a, q4b = oo
                        act_at.setdefault(q4a[1], []).append(("p1", q4a))
                        act_at.setdefault(q4a[3], []).append(("p3h", q4a))
                        act_at.setdefault(q4b[1], []).append(("p1", q4b))
                        act_at.setdefault(q4b[3], []).append(("p3o", oo))
                    for pp in pairs:
                        act_at.setdefault(pp[1], []).append(("pair", pp))
                    for ss in singles:
                        act_at.setdefault(ss, []).append(("single", ss))

                    # o-proj filler positions: one prev-qc iblk emitted inside
                    # the j loop every ~nkb/4 blocks (phase B is exp-chain
                    # paced; the independent o-proj matmuls absorb the PE
                    # waits at block granularity)
                    fill_js = {max(0, (i + 1) * nkb // 4 - 2): i for i in range(4)}

                    for h in range(HPC):
                        # flush the deferred tail first: the in-loop o-proj
                        # filler needs the previous qc's LAST attn tile
                        if tail_state is not None:
                            emit_tail(tail_state)
                            tail_state = None
                        ps_att = psB_a.tile([128, 512], f32, tag="att")
                        ps_den = psB_d.tile([128, 512], f32, tag="d")
                        qtile = q_tiles[(h, qc)]
                        prs_h = [None] * nkb
                        den_first = [True]
                        qsums = {}
                        mmq = []

                        def den_mm(rhs, offp, stop):
                            nc.tensor.matmul(ps_den[:, offp:], onesq_r[:],
                                             rhs[:, offp:],
                                             start=den_first[0], stop=stop)
                            den_first[0] = False

                        def den_step(jp):
                            for act in act_at.get(jp, ()):
                                kind = act[0]
                                if kind == "p1":
                                    q4 = act[1]
                                    s1 = dsum_pool.tile([128, 512], bf16, tag="ds")
                                    nc.vector.tensor_add(s1[:], prs_h[q4[0]][:],
                                                         prs_h[q4[1]][:])
                                    qsums[q4] = s1
                                elif kind in ("p3", "p3h", "p3o"):
                                    oo = act[1] if kind == "p3o" else None
                                    q4 = oo[1] if oo else act[1]
                                    s2 = dsum_pool.tile([128, 512], bf16, tag="ds")
                                    nc.vector.tensor_add(s2[:], prs_h[q4[2]][:],
                                                         prs_h[q4[3]][:])
                                    qd = dsum_pool.tile([128, 512], bf16, tag="ds")
                                    nc.vector.tensor_add(qd[:], qsums.pop(q4)[:],
                                                         s2[:])
                                    if kind == "p3":
                                        mmq.append((jp + 2, qd, 0))
                                    elif kind == "p3h":
                                        qsums[("held", q4)] = qd
                                    else:
                                        od = dsum_pool.tile([128, 512], bf16, tag="ds")
                                        nc.vector.tensor_add(
                                            od[:], qsums.pop(("held", oo[0]))[:],
                                            qd[:])
                                        mmq.append((jp + 3, od, 0))
                                elif kind == "pair":
                                    pp = act[1]
                                    ds = dsum_pool.tile([128, 512], bf16, tag="ds")
                                    nc.vector.tensor_add(ds[:], prs_h[pp[0]][:],
                                                         prs_h[pp[1]][:])
                                    mmq.append((jp + 1, ds, 0))
                                else:
                                    mmq.append((jp + 1, prs_h[jp], kbs[jp][2]))
                            while mmq and mmq[0][0] <= jp:
                                _, rhs, offp = mmq.pop(0)
                                den_mm(rhs, offp, False)

                        pend = None
                        for j, (kb, mi, off) in enumerate(kbs):
                            ps_s = psB_s.tile([128, 512], f32, tag="s")
                            nc.tensor.matmul(
                                ps_s[:, off:],
                                kt[h][kb // 4][:, (kb % 4) * 128:(kb % 4 + 1) * 128],
                                qtile[:, off:], start=True, stop=True)
                            if mi is not None:
                                msb = mask_sb[mi] if preload_masks else None
                                if msb is None:
                                    msb = mp.tile([128, 512], bf16, tag="mk", name=f"mks{mi}")
                                    nc.gpsimd.dma_start(out=msb, in_=mask_d[mi, :, :])
                                # psum-read sbuf-write: in-place psum add would
                                # halve DVE rate (single psum port, read+write)
                                sm = smx_pool.tile([128, 512], f32, tag="sm")
                                nc.vector.tensor_add(sm[:, off:], ps_s[:, off:],
                                                     msb[:, off:])
                                exp_src = sm
                            else:
                                exp_src = ps_s
                            pr = probs_pool.tile([128, 512], bf16, tag="pr")
                            nc.scalar.activation(pr[:, off:], exp_src[:, off:], Exp,
                                                 bias=shiftb[:], scale=float(SCALE))
                            prs_h[j] = pr
                            # defer pv one kb (and den two) so the next score
                            # matmul keeps PE fed while ACT finishes exp(j)
                            if pend is not None:
                                jp, kbp, prp, offp = pend
                                nc.tensor.matmul(ps_att[:, offp:],
                                                 v_tiles[kbp][:, h * 128:(h + 1) * 128],
                                                 prp[:, offp:],
                                                 start=(jp == 0), stop=False)
                                den_step(jp)
                            if qc > 0 and j in fill_js:
                                emit_c_chunk(qc - 1, [4 * h + fill_js[j]])
                            pend = (j, kb, pr, off)
                        jp, kbp, prp, offp = pend
                        nc.tensor.matmul(ps_att[:, offp:],
                                         v_tiles[kbp][:, h * 128:(h + 1) * 128],
                                         prp[:, offp:], start=(jp == 0), stop=True)
                        den_step(jp)
                        while mmq:
                            _, rhs, offp2 = mmq.pop(0)
                            den_mm(rhs, offp2, stop=(not mmq))
                        # evict den now (frees the den psum bank for next h);
                        # the rest of the tail is deferred one head for pipelining
                        if qc == 3 and h == HPC - 1:
                            # last tail: reciprocal straight from PSUM; the
                            # den bank has no next user and skipping the ACT
                            # copy shortens the final attn chain gating the
                            # closing o-proj chunks
                            tail_state = (h, qc, ps_att, ps_den)
                        else:
                            den_sb = den_pool.tile([128, 512], f32, tag="dn")
                            nc.scalar.copy(out=den_sb[:], in_=ps_den[:])
                            tail_state = (h, qc, ps_att, den_sb)
                    if qc > 0:
                        for hh in range(HPC):
                            del attn_tiles[(hh, qc - 1)]
                emit_tail(tail_state)
                emit_c_chunk(3, range(16))
                for hh in range(HPC):
                    del attn_tiles[(hh, 3)]

    nc.compile()
    return nc


LAST_EXEC_NS = None


def kernel(hidden_states, Wq, Wk, Wv, Wo, attention_mask):
    global LAST_EXEC_NS
    from concourse.bass_utils import run_bass_kernel_spmd

    hidden_states = np.asarray(hidden_states, dtype=np.float32)
    Wq = np.asarray(Wq, dtype=np.float32)
    Wk = np.asarray(Wk, dtype=np.float32)
    Wv = np.asarray(Wv, dtype=np.float32)
    Wo = np.asarray(Wo, dtype=np.float32)
    attention_mask = np.asarray(attention_mask, dtype=np.float32)

    cosT, ssinT = _rope_tables()
    plan, mtiles = _mask_plan(attention_mask[0])
    nm = len(mtiles)
    preload = nm <= 24
    maskt = np.stack(mtiles).astype(BF16) if nm else None

    plan_key = (tuple(tuple(r) for r in plan), nm, preload)
    nc = _PROGRAM_CACHE.get(plan_key)
    if nc is None:
        nc = _build_program(plan, nm, preload)
        _PROGRAM_CACHE[plan_key] = nc

    perm = np.concatenate([np.arange(0, HD, 2), np.arange(1, HD, 2)])
    Wq4 = Wq.reshape(NH, HD, HID)[:, perm, :]
    Wk4 = Wk.reshape(NH, HD, HID)[:, perm, :]
    Wv4 = Wv.reshape(NH, HD, HID)

    # [4, 128, 16, 512] per-partition-contiguous hs tiling, per batch (bf16)
    hs_tl = [np.ascontiguousarray(
        hidden_states[b].reshape(4, 512, 16, 128).transpose(0, 3, 2, 1)).astype(BF16)
        for b in range(B)]

    def tile_qk(mT):   # [HID, 512] -> [128, 16, 4, 128]
        return np.ascontiguousarray(
            mT.reshape(16, 128, 4, 128).transpose(1, 0, 2, 3)).astype(BF16)

    in_maps = []
    for c in range(NCORES):
        b, hg = divmod(c, HPC)
        heads = slice(hg * HPC, (hg + 1) * HPC)
        wqT = Wq4[heads].reshape(512, HID).T          # [HID, 512]
        wkT = Wk4[heads].reshape(512, HID).T
        wvT = Wv4[heads].reshape(512, HID).T          # [HID, 512]
        woT = Wo[:, hg * 512:(hg + 1) * 512].T        # [512, HID]
        m = {
            "hs_t": hs_tl[b],
            "wq_t": tile_qk(wqT),
            "wk_t": tile_qk(wkT),
            "wv_t": np.ascontiguousarray(
                wvT.reshape(16, 128, 512).transpose(1, 0, 2)).astype(BF16),
            "wo_t": np.ascontiguousarray(
                woT.reshape(4, 128, 16, 128).transpose(1, 0, 2, 3)).astype(BF16),
            "cosT": cosT.astype(BF16),
            "ssinT": ssinT.astype(BF16),
        }
        if nm:
            m["maskt"] = maskt
        in_maps.append(m)

    trace = bool(os.environ.get("CC_BASS_TRACE"))
    res = run_bass_kernel_spmd(nc, in_maps, core_ids=list(range(NCORES)), trace=trace)
    LAST_EXEC_NS = res.exec_time_ns

    out = np.empty((B, S, S), dtype=np.float32)
    for b in range(B):
        acc = res.results[b * HPC]["oT_t"].astype(np.float32)
        for hg in range(1, HPC):
            acc = acc + res.results[b * HPC + hg]["oT_t"].astype(np.float32)
        # [qc, iblk, p, t] -> [iblk*128+p, qc*512+t] = oT_full, out = oT_full.T
        o_full = acc.transpose(1, 2, 0, 3).reshape(S, S)
        out[b] = o_full.T
    return out


# revision 42
# speedup vs baseline: 1.1070x; 1.1070x over previous
"""Evo2 attention (B=2, S=2048, HID=2048, NH=16, HD=128) on 8 trn2 NeuronCores.

Sharding: core c handles batch b=c//4 and heads 4*(c%4)..4*(c%4)+3.
Megatron-style: q/k/v projections column-parallel, o_proj row-parallel with the
4-way partial sum done on host during unshard.

Per-core kernel layout (everything transposed so no on-chip transposes needed):
  hsT [hid, tok] -> qT,kT [hd, tok] (RoPE fused into PSUM eviction, rotate-half
  basis obtained by de-interleaving W rows on host), v [tok, hd].
  scoresT[k, q] = kT_blk vs qT matmul; softmax over k (= partitions) with a
  fixed shift instead of a max; denominators via ones-vector PE reduction and
  a K=1 matmul broadcast; PV gives attnT [hd, q]; o_projT partial [o, q].

Optimizations vs the fp32r baseline (429us -> ~339us):
  - all matmul operands bf16 (PSUM accumulation stays fp32): halves DMA+SBUF,
    enables FWL weight loads; output partials bf16, summed fp32 on host
  - phase A: one hs residency per 512-token chunk feeds q, k AND v passes
    (hs loaded once, not twice); dblk-outer groups so rope evictions stagger;
    weights DMA'd in 2-kc pieces interleaved with hs tiles (short head)
  - PE warmup matmuls during the initial DMA fill (HAM un-throttle) and an
    early dummy exp (ACT table-set preload off the critical path)
  - causal diagonal blocks column-trimmed: fully-masked leading q-columns of
    each [128k x 512q] tile are never computed (score/exp/den/PV all skip)
  - reciprocal -> single-pass reciprocal_approx_fast (~5x faster DVE op)
  - denominator reductions: off-diagonal probs tiles summed pairwise on DVE
    (bf16 2x) so half those N=512 PE streams disappear
  - dependency-granularity fixes: per-(head,chunk) kt tiles, perm-scoped mask
    pool with DMAs issued after the weight pieces (Tile deps are per-tile and
    pool SBUF ranges alias phase A's, which otherwise serializes phase B)
  - o-projection of the previous q-chunk interleaved INTO the score/PV loop
    at block granularity as PE filler for the exp-chain waits
"""
import os
import sys
import numpy as np

for _p in ("/opt/trn_rl_repo",):
    if os.path.isdir(_p) and _p not in sys.path:
        sys.path.insert(0, _p)

import ml_dtypes

BF16 = ml_dtypes.bfloat16
B, S, HID, NH = 2, 2048, 2048, 16
HD = HID // NH            # 128
HPC = 4                   # heads per core
NCORES = 8
BASE = 10000.0
SCALE = 1.0 / np.sqrt(HD).astype(np.float32)
SHIFT = 25.0              # fixed softmax shift (replaces per-row max)
NEG_INF_THRESH = -1e8

_PROGRAM_CACHE = {}


def _rope_tables():
    """cos/±sin tables [HD, S] in the de-interleaved (rotate-half) basis.

    Reference pairs dims (2m, 2m+1) with angle theta_m(s) = s * inv_freq[f(m)],
    f(m) = 2m for m<32 else 2m-64 (from emb[:, ::2] of concat([freqs, freqs])).
    After de-interleave perm [0,2,..126,1,3,..127]: new dim m<64 is old 2m,
    new dim 64+m is old 2m+1.
      out[m]    = x[m] cos_m - x[64+m] sin_m
      out[64+m] = x[m] sin_m + x[64+m] cos_m
    """
    inv_freq = BASE ** (-np.arange(0, HD, 2, dtype=np.float64) / HD)  # [64]
    m = np.arange(64)
    fmap = np.where(m < 32, 2 * m, 2 * m - 64)
    t = np.arange(S, dtype=np.float64)
    theta = t[None, :] * inv_freq[fmap][:, None]          # [64, S]
    cos = np.cos(theta)
    sin = np.sin(theta)
    cosT = np.concatenate([cos, cos], axis=0).astype(np.float32)      # [128, S]
    # row d holds the factor applied to SOURCE half d (dest = other half):
    # src lo -> dst hi uses +sin; src hi -> dst lo uses -sin
    ssinT = np.concatenate([sin, -sin], axis=0).astype(np.float32)    # [128, S]
    return cosT, ssinT


def _mask_plan(mask2d):
    """Classify [128k x 512q] blocks of mask^T. Returns (plan, tiles).

    plan[qc] = list of (kb, mask_tile_idx_or_None, col_off); fully-masked
    blocks skipped; col_off = count of leading fully-masked q-columns (those
    columns are skipped entirely in score/exp/den/PV).
    tiles: deduped f32 [128, 512] mask^T blocks prescaled by sqrt(HD).
    """
    maskT = np.ascontiguousarray(mask2d.T)  # [k, q]
    plan = []
    tiles = []
    seen = {}
    for qc in range(S // 512):
        row = []
        for kb in range(S // 128):
            sub = maskT[kb * 128:(kb + 1) * 128, qc * 512:(qc + 1) * 512]
            if (sub <= NEG_INF_THRESH).all():
                continue
            colmasked = (sub <= NEG_INF_THRESH).all(axis=0)  # [512]
            off = 0
            while off < 512 and colmasked[off]:
                off += 1
            if (sub[:, off:] == 0.0).all():
                row.append((kb, None, off))
                continue
            pre = np.ascontiguousarray(sub * np.float32(np.sqrt(HD)))
            key = pre.tobytes()
            idx = seen.get(key)
            if idx is None:
                idx = len(tiles)
                seen[key] = idx
                tiles.append(pre)
            row.append((kb, idx, off))
        plan.append(row)
    return plan, tiles


def _build_program(plan, nm, preload_masks):
    import contextlib
    import concourse.bacc as bacc
    import concourse.tile as tile
    from concourse import mybir

    f32 = mybir.dt.float32
    f32r = mybir.dt.float32r
    bf16 = mybir.dt.bfloat16
    nc = bacc.Bacc(None, target_bir_lowering=False)

    # host-pre-tiled inputs: last axis group per partition is contiguous
    hs_d = nc.dram_tensor("hs_t", [4, 128, 16, 512], bf16, kind="ExternalInput")
    wq_d = nc.dram_tensor("wq_t", [128, 16, 4, 128], bf16, kind="ExternalInput")
    wk_d = nc.dram_tensor("wk_t", [128, 16, 4, 128], bf16, kind="ExternalInput")
    wv_d = nc.dram_tensor("wv_t", [128, 16, 512], bf16, kind="ExternalInput")
    wo_d = nc.dram_tensor("wo_t", [128, 4, 16, 128], bf16, kind="ExternalInput")
    cos_d = nc.dram_tensor("cosT", [128, S], bf16, kind="ExternalInput")
    sin_d = nc.dram_tensor("ssinT", [128, S], bf16, kind="ExternalInput")
    if nm:
        mask_d = nc.dram_tensor("maskt", [nm, 128, 512], bf16, kind="ExternalInput")
    o_d = nc.dram_tensor("oT_t", [4, 16, 128, 512], bf16, kind="ExternalOutput")

    Exp = mybir.ActivationFunctionType.Exp

    with tile.TileContext(nc) as tc:
        with contextlib.ExitStack() as perm:
            kt_pool = perm.enter_context(tc.tile_pool(name="kt", bufs=16))
            qa_pool = perm.enter_context(tc.tile_pool(name="qa", bufs=17))
            cst = perm.enter_context(tc.tile_pool(name="cst", bufs=1))
            tmp_pool = perm.enter_context(tc.tile_pool(name="tmp", bufs=2))
            wo_pool = perm.enter_context(tc.tile_pool(name="wo", bufs=1))

            # ---- PE warmup: ~3.5us of dummy matmuls while DMAs fill ------
            wup_st = cst.tile([128, 512], f32, tag="wu0")
            nc.vector.memset(wup_st, 0.0)
            wup = cst.tile([128, 512], bf16, tag="wu1")
            nc.vector.tensor_copy(out=wup, in_=wup_st)

            onesq_st = cst.tile([128, 128], f32, tag="o3")
            nc.vector.memset(onesq_st, 1.0)
            onesq_r = cst.tile([128, 128], bf16, tag="o4")
            nc.vector.tensor_copy(out=onesq_r, in_=onesq_st)
            ones_r1 = cst.tile([128, 128], f32r, tag="o5")
            nc.vector.tensor_copy(out=ones_r1[0:1, :], in_=onesq_st[0:1, :])
            shiftb = cst.tile([128, 1], f32, tag="sh")
            nc.vector.memset(shiftb, -SHIFT)
            # preload the exp ACT table set now; otherwise the first phase-B
            # exp pays the ~2.7us ACT_TABLE_LOAD + DRAIN on the critical path
            expwarm = cst.tile([128, 1], f32, tag="ew")
            nc.scalar.activation(expwarm[:], shiftb[:], Exp)

            # per-(head, chunk) kt tiles: tile-granular dependency tracking
            # means one big [128,S] tile would gate phase B's first score
            # matmul on the LAST chunk's rope writes
            kt = [[kt_pool.tile([128, 512], bf16, tag="kt", name=f"kt{i}_{c}")
                   for c in range(4)] for i in range(HPC)]

            # mask tiles live in a perm-scoped pool: allocated inside phase
            # B's stack their SBUF range overlaps phase A's pools, which makes
            # the DMA wait for ALL phase-A matmuls and puts ~5us of mask
            # latency on the first exp chain. The DMAs themselves are issued
            # later (after the weight pieces) so they don't delay the head.
            mask_sb = None
            if nm and preload_masks:
                mkp = perm.enter_context(tc.tile_pool(name="mk", bufs=nm))
                mask_sb = [mkp.tile([128, 512], bf16, tag="mk", name=f"mk{i}")
                           for i in range(nm)]
            v_tiles = [None] * 16
            q_tiles = {}
            attn_tiles = {}

            def rope_evict(ps, dst, cos_sb, sin_sb):
                nc.vector.tensor_mul(dst[0:64, :], ps[64:128, :], sin_sb[64:128, :])
                nc.vector.tensor_mul(dst[64:128, :], ps[0:64, :], sin_sb[0:64, :])
                t = tmp_pool.tile([128, 512], f32, tag="ropetmp")
                nc.vector.tensor_mul(t[:], ps[:], cos_sb[:, :])
                nc.vector.tensor_add(dst[:, :], dst[:, :], t[:])

            # ---- Phase A: projections. One hs residency per 512-token
            # chunk feeds the q, k AND v passes (hs loaded once). kc-outer
            # loop order: the first matmuls gate only on the first kc slice
            # of weights + first hs tile.
            with contextlib.ExitStack() as actx:
                hs_pool = actx.enter_context(tc.tile_pool(name="hs", bufs=8))
                tabp = actx.enter_context(tc.tile_pool(name="tab", bufs=4))
                w_pool = actx.enter_context(tc.tile_pool(name="w", bufs=1))
                # 4+3 = 7 banks: one PSUM bank stays free so phase B's first
                # score matmul isn't gated on phase A's full eviction chain
                psA = actx.enter_context(tc.tile_pool(name="psA", bufs=4, space="PSUM"))
                psAv = actx.enter_context(tc.tile_pool(name="psAv", bufs=3, space="PSUM"))
                v_pool = perm.enter_context(tc.tile_pool(name="v", bufs=16, side="right"))

                def load_hs(c):
                    qts = []
                    for hf in range(4):
                        ht = hs_pool.tile([128, 4, 512], bf16, tag="hs",
                                          name=f"hs{c}_{hf}")
                        nc.sync.dma_start(
                            out=ht, in_=hs_d[c, :, hf * 4:(hf + 1) * 4, :])
                        qts.append(ht)
                    return qts

                # warmup matmuls fill the PE while the first DMAs land
                # (~12us of dummy work: HAM un-throttles and PE is hot when
                # the first real operands arrive)
                ps_wu = psA.tile([128, 512], f32, tag="qk", name="pswu")
                for i in range(56):
                    nc.tensor.matmul(ps_wu[:], wup[:, 0:128], wup[:],
                                     start=True, stop=True)

                wq_all = w_pool.tile([128, 16, 4, 128], bf16, tag="wq", name="wqall")
                wk_all = w_pool.tile([128, 16, 4, 128], bf16, tag="wk", name="wkall")
                wv_all = w_pool.tile([128, 16, 512], bf16, tag="wv", name="wvall")
                wo_all = wo_pool.tile([128, 4, 16, 128], bf16, tag="wo", name="woall")
                # interleave hs chunk-0 tiles with wq pieces in consumption
                # order: the first accumulation group reads (ht[kc//4], wq[kc])
                # kc-ascending, so the head is gated on ~0.8MB, not 4MB
                hs_cur = []
                for hf in range(4):
                    ht = hs_pool.tile([128, 4, 512], bf16, tag="hs", name=f"hs0_{hf}")
                    nc.sync.dma_start(out=ht, in_=hs_d[0, :, hf * 4:(hf + 1) * 4, :])
                    hs_cur.append(ht)
                    for p in (2 * hf, 2 * hf + 1):
                        nc.sync.dma_start(out=wq_all[:, p * 2:(p + 1) * 2, :, :],
                                          in_=wq_d[:, p * 2:(p + 1) * 2, :, :])
                for p in range(8):
                    nc.sync.dma_start(out=wk_all[:, p * 2:(p + 1) * 2, :, :],
                                      in_=wk_d[:, p * 2:(p + 1) * 2, :, :])
                for p in range(4):
                    nc.sync.dma_start(out=wv_all[:, p * 4:(p + 1) * 4, :],
                                      in_=wv_d[:, p * 4:(p + 1) * 4, :])
                for p in range(4):
                    nc.sync.dma_start(out=wo_all[:, p, :, :],
                                      in_=wo_d[:, p, :, :])
                if mask_sb is not None:
                    for i in range(nm):
                        nc.sync.dma_start(out=mask_sb[i], in_=mask_d[i, :, :])
                for c in range(4):
                    hs_nxt = load_hs(c + 1) if c < 3 else None
                    cos_sb = tabp.tile([128, 512], bf16, tag="cos")
                    nc.gpsimd.dma_start(out=cos_sb, in_=cos_d[:, c * 512:(c + 1) * 512])
                    sin_sb = tabp.tile([128, 512], bf16, tag="sin")
                    nc.gpsimd.dma_start(out=sin_sb, in_=sin_d[:, c * 512:(c + 1) * 512])
                    # q pass then k pass (dblk-outer: evictions stagger so the
                    # rope DVE work overlaps the next accumulation group)
                    for pass_i, w_all in ((0, wq_all), (1, wk_all)):
                        for dblk in range(4):
                            ps = psA.tile([128, 512], f32, tag="qk",
                                          name=f"ps{pass_i}_{c}_{dblk}")
                            for kc in range(16):
                                nc.tensor.matmul(
                                    ps[:], w_all[:, kc, dblk, :],
                                    hs_cur[kc // 4][:, kc % 4, :],
                                    start=(kc == 0), stop=(kc == 15))
                            if pass_i == 0:
                                q = qa_pool.tile([128, 512], bf16, tag="qa")
                                rope_evict(ps, q, cos_sb, sin_sb)
                                q_tiles[(dblk, c)] = q
                            else:
                                rope_evict(ps[:, :], kt[dblk][c][:, :],
                                           cos_sb, sin_sb)
                    # v pass (vblk-outer, evictions stagger onto ACT)
                    for vblk in range(4):
                        psv = psAv.tile([128, 512], f32, tag="v",
                                        name=f"psv{c}_{vblk}")
                        for kc in range(16):
                            nc.tensor.matmul(
                                psv[:],
                                hs_cur[kc // 4][:, kc % 4, vblk * 128:(vblk + 1) * 128],
                                wv_all[:, kc, :], start=(kc == 0), stop=(kc == 15))
                        vt = v_pool.tile([128, 512], bf16, tag="v", name=f"v{c}_{vblk}")
                        nc.scalar.copy(out=vt[:], in_=psv[:])
                        v_tiles[c * 4 + vblk] = vt
                    hs_cur = hs_nxt

            # ---------------- Phase B + C ------------------------------------
            with contextlib.ExitStack() as bctx:
                probs_pool = bctx.enter_context(tc.tile_pool(name="pr", bufs=12))
                dsum_pool = bctx.enter_context(tc.tile_pool(name="ds", bufs=8))
                smx_pool = bctx.enter_context(tc.tile_pool(name="sm", bufs=6))
                den_pool = bctx.enter_context(tc.tile_pool(name="dn", bufs=4))
                rcb_pool = bctx.enter_context(tc.tile_pool(name="rcb", bufs=3))
                outb_pool = bctx.enter_context(tc.tile_pool(name="ob", bufs=4))
                psB_s = bctx.enter_context(tc.tile_pool(name="psBs", bufs=3, space="PSUM"))
                psB_a = bctx.enter_context(tc.tile_pool(name="psBa", bufs=2, space="PSUM"))
                psB_d = bctx.enter_context(tc.tile_pool(name="psBd", bufs=1, space="PSUM"))
                psC = bctx.enter_context(tc.tile_pool(name="psC", bufs=2, space="PSUM"))
                if nm and not preload_masks:
                    mp = bctx.enter_context(tc.tile_pool(name="mk", bufs=8))

                def emit_tail(state):
                    h, qc, ps_att, den_sb = state
                    rcb = rcb_pool.tile([128, 512], f32, tag="rcb")
                    nc.vector.reciprocal_approx_fast(out=rcb[:], in_=den_sb[:])
                    at = qa_pool.tile([128, 512], bf16, tag="qa")
                    nc.vector.tensor_mul(at[:], ps_att[:], rcb[:])
                    attn_tiles[(h, qc)] = at

                def emit_c_chunk(qc, iblks):
                    for iblk in iblks:
                        ps_o = psC.tile([128, 512], f32, tag="o")
                        for jc in range(4):
                            nc.tensor.matmul(ps_o[:], wo_all[:, jc, iblk, :],
                                             attn_tiles[(jc, qc)][:],
                                             start=(jc == 0), stop=(jc == 3))
                        ob = outb_pool.tile([128, 512], bf16, tag="ob")
                        if iblk % 8 < 5:
                            nc.vector.tensor_copy(out=ob[:], in_=ps_o[:])
                        else:
                            nc.scalar.copy(out=ob[:], in_=ps_o[:])
                        nc.sync.dma_start(out=o_d[qc, iblk, :, :], in_=ob[:])

                tail_state = None
                for qc in range(4):
                    kbs = plan[qc]
                    nkb = len(kbs)
                    # den schedule: off==0 blocks group into quads (two DVE
                    # bf16 adds collapse 4 probs tiles into ONE N=512 PE
                    # reduction stream); leftovers pair; diagonals go single.
                    # Den matmuls are emitted a couple of blocks late so the
                    # DVE adds are off the PE's critical path.
                    off0 = [jj for jj, e in enumerate(kbs) if e[2] == 0]
                    diag = [jj for jj, e in enumerate(kbs) if e[2] != 0]
                    quads, rest = [], list(off0)
                    while len(rest) >= 4:
                        quads.append(tuple(rest[:4]))
                        rest = rest[4:]
                    octs, qrest = [], list(quads)
                    while len(qrest) >= 2:
                        octs.append((qrest[0], qrest[1]))
                        qrest = qrest[2:]
                    quads = qrest
                    pairs = []
                    if len(rest) >= 2:
                        pairs.append((rest[0], rest[1]))
                        rest = rest[2:]
                    singles = rest + diag
                    act_at = {}
                    for q4 in quads:
                        act_at.setdefault(q4[1], []).append(("p1", q4))
                        act_at.setdefault(q4[3], []).append(("p3", q4))
                    for oo in octs:
                        q4a, q4b = oo
                        act_at.setdefault(q4a[1], []).append(("p1", q4a))
                        act_at.setdefault(q4a[3], []).append(("p3h", q4a))
                        act_at.setdefault(q4b[1], []).append(("p1", q4b))
                        act_at.setdefault(q4b[3], []).append(("p3o", oo))
                    for pp in pairs:
                        act_at.setdefault(pp[1], []).append(("pair", pp))
                    for ss in singles:
                        act_at.setdefault(ss, []).append(("single", ss))

                    # o-proj filler positions: one prev-qc iblk emitted inside
                    # the j loop every ~nkb/4 blocks (phase B is exp-chain
                    # paced; the independent o-proj matmuls absorb the PE
                    # waits at block granularity)
                    fill_js = {(i + 1) * nkb // 4 - 1: i for i in range(4)}

                    for h in range(HPC):
                        # flush the deferred tail first: the in-loop o-proj
                        # filler needs the previous qc's LAST attn tile
                        if tail_state is not None:
                            emit_tail(tail_state)
                            tail_state = None
                        ps_att = psB_a.tile([128, 512], f32, tag="att")
                        ps_den = psB_d.tile([128, 512], f32, tag="d")
                        qtile = q_tiles[(h, qc)]
                        prs_h = [None] * nkb
                        den_first = [True]
                        qsums = {}
                        mmq = []

                        def den_mm(rhs, offp, stop):
                            nc.tensor.matmul(ps_den[:, offp:], onesq_r[:],
                                             rhs[:, offp:],
                                             start=den_first[0], stop=stop)
                            den_first[0] = False

                        def den_step(jp):
                            for act in act_at.get(jp, ()):
                                kind = act[0]
                                if kind == "p1":
                                    q4 = act[1]
                                    s1 = dsum_pool.tile([128, 512], bf16, tag="ds")
                                    nc.vector.tensor_add(s1[:], prs_h[q4[0]][:],
                                                         prs_h[q4[1]][:])
                                    qsums[q4] = s1
                                elif kind in ("p3", "p3h", "p3o"):
                                    oo = act[1] if kind == "p3o" else None
                                    q4 = oo[1] if oo else act[1]
                                    s2 = dsum_pool.tile([128, 512], bf16, tag="ds")
                                    nc.vector.tensor_add(s2[:], prs_h[q4[2]][:],
                                                         prs_h[q4[3]][:])
                                    qd = dsum_pool.tile([128, 512], bf16, tag="ds")
                                    nc.vector.tensor_add(qd[:], qsums.pop(q4)[:],
                                                         s2[:])
                                    if kind == "p3":
                                        mmq.append((jp + 2, qd, 0))
                                    elif kind == "p3h":
                                        qsums[("held", q4)] = qd
                                    else:
                                        od = dsum_pool.tile([128, 512], bf16, tag="ds")
                                        nc.vector.tensor_add(
                                            od[:], qsums.pop(("held", oo[0]))[:],
                                            qd[:])
                                        mmq.append((jp + 3, od, 0))
                                elif kind == "pair":
                                    pp = act[1]
                                    ds = dsum_pool.tile([128, 512], bf16, tag="ds")
                                    nc.vector.tensor_add(ds[:], prs_h[pp[0]][:],
                                                         prs_h[pp[1]][:])
                                    mmq.append((jp + 1, ds, 0))
                                else:
                                    mmq.append((jp + 1, prs_h[jp], kbs[jp][2]))
                            while mmq and mmq[0][0] <= jp:
                                _, rhs, offp = mmq.pop(0)
                                den_mm(rhs, offp, False)

                        pend = None
                        for j, (kb, mi, off) in enumerate(kbs):
                            ps_s = psB_s.tile([128, 512], f32, tag="s")
                            nc.tensor.matmul(
                                ps_s[:, off:],
                                kt[h][kb // 4][:, (kb % 4) * 128:(kb % 4 + 1) * 128],
                                qtile[:, off:], start=True, stop=True)
                            if mi is not None:
                                msb = mask_sb[mi] if preload_masks else None
                                if msb is None:
                                    msb = mp.tile([128, 512], bf16, tag="mk", name=f"mks{mi}")
                                    nc.gpsimd.dma_start(out=msb, in_=mask_d[mi, :, :])
                                # psum-read sbuf-write: in-place psum add would
                                # halve DVE rate (single psum port, read+write)
                                sm = smx_pool.tile([128, 512], f32, tag="sm")
                                nc.vector.tensor_add(sm[:, off:], ps_s[:, off:],
                                                     msb[:, off:])
                                exp_src = sm
                            else:
                                exp_src = ps_s
                            pr = probs_pool.tile([128, 512], bf16, tag="pr")
                            nc.scalar.activation(pr[:, off:], exp_src[:, off:], Exp,
                                                 bias=shiftb[:], scale=float(SCALE))
                            prs_h[j] = pr
                            # defer pv one kb (and den two) so the next score
                            # matmul keeps PE fed while ACT finishes exp(j)
                            if pend is not None:
                                jp, kbp, prp, offp = pend
                                nc.tensor.matmul(ps_att[:, offp:],
                                                 v_tiles[kbp][:, h * 128:(h + 1) * 128],
                                                 prp[:, offp:],
                                                 start=(jp == 0), stop=False)
                                den_step(jp)
                            if qc > 0 and j in fill_js:
                                emit_c_chunk(qc - 1, [4 * h + fill_js[j]])
                            pend = (j, kb, pr, off)
                        jp, kbp, prp, offp = pend
                        nc.tensor.matmul(ps_att[:, offp:],
                                         v_tiles[kbp][:, h * 128:(h + 1) * 128],
                                         prp[:, offp:], start=(jp == 0), stop=True)
                        den_step(jp)
                        while mmq:
                            _, rhs, offp2 = mmq.pop(0)
                            den_mm(rhs, offp2, stop=(not mmq))
                        # evict den now (frees the den psum bank for next h);
                        # the rest of the tail is deferred one head for pipelining
                        if qc == 3 and h == HPC - 1:
                            # last tail: reciprocal straight from PSUM; the
                            # den bank has no next user and skipping the ACT
                            # copy shortens the final attn chain gating the
                            # closing o-proj chunks
                            tail_state = (h, qc, ps_att, ps_den)
                        else:
                            den_sb = den_pool.tile([128, 512], f32, tag="dn")
                            nc.scalar.copy(out=den_sb[:], in_=ps_den[:])
                            tail_state = (h, qc, ps_att, den_sb)
                    if qc > 0:
                        for hh in range(HPC):
                            del attn_tiles[(hh, qc - 1)]
                emit_tail(tail_state)
                emit_c_chunk(3, range(16))
                for hh in range(HPC):
                    del attn_tiles[(hh, 3)]

    nc.compile()
    return nc


LAST_EXEC_NS = None


def kernel(hidden_states, Wq, Wk, Wv, Wo, attention_mask):
    global LAST_EXEC_NS
    from concourse.bass_utils import run_bass_kernel_spmd

    hidden_states = np.asarray(hidden_states, dtype=np.float32)
    Wq = np.asarray(Wq, dtype=np.float32)
    Wk = np.asarray(Wk, dtype=np.float32)
    Wv = np.asarray(Wv, dtype=np.float32)
    Wo = np.asarray(Wo, dtype=np.float32)
    attention_mask = np.asarray(attention_mask, dtype=np.float32)

    cosT, ssinT = _rope_tables()
    plan, mtiles = _mask_plan(attention_mask[0])
    nm = len(mtiles)
    preload = nm <= 24
    maskt = np.stack(mtiles).astype(BF16) if nm else None

    plan_key = (tuple(tuple(r) for r in plan), nm, preload)
    nc = _PROGRAM_CACHE.get(plan_key)
    if nc is None:
        nc = _build_program(plan, nm, preload)
        _PROGRAM_CACHE[plan_key] = nc

    perm = np.concatenate([np.arange(0, HD, 2), np.arange(1, HD, 2)])
    Wq4 = Wq.reshape(NH, HD, HID)[:, perm, :]
    Wk4 = Wk.reshape(NH, HD, HID)[:, perm, :]
    Wv4 = Wv.reshape(NH, HD, HID)

    # [4, 128, 16, 512] per-partition-contiguous hs tiling, per batch (bf16)
    hs_tl = [np.ascontiguousarray(
        hidden_states[b].reshape(4, 512, 16, 128).transpose(0, 3, 2, 1)).astype(BF16)
        for b in range(B)]

    def tile_qk(mT):   # [HID, 512] -> [128, 16, 4, 128]
        return np.ascontiguousarray(
            mT.reshape(16, 128, 4, 128).transpose(1, 0, 2, 3)).astype(BF16)

    in_maps = []
    for c in range(NCORES):
        b, hg = divmod(c, HPC)
        heads = slice(hg * HPC, (hg + 1) * HPC)
        wqT = Wq4[heads].reshape(512, HID).T          # [HID, 512]
        wkT = Wk4[heads].reshape(512, HID).T
        wvT = Wv4[heads].reshape(512, HID).T          # [HID, 512]
        woT = Wo[:, hg * 512:(hg + 1) * 512].T        # [512, HID]
        m = {
            "hs_t": hs_tl[b],
            "wq_t": tile_qk(wqT),
            "wk_t": tile_qk(wkT),
            "wv_t": np.ascontiguousarray(
                wvT.reshape(16, 128, 512).transpose(1, 0, 2)).astype(BF16),
            "wo_t": np.ascontiguousarray(
                woT.reshape(4, 128, 16, 128).transpose(1, 0, 2, 3)).astype(BF16),
            "cosT": cosT.astype(BF16),
            "ssinT": ssinT.astype(BF16),
        }
        if nm:
            m["maskt"] = maskt
        in_maps.append(m)

    trace = bool(os.environ.get("CC_BASS_TRACE"))
    res = run_bass_kernel_spmd(nc, in_maps, core_ids=list(range(NCORES)), trace=trace)
    LAST_EXEC_NS = res.exec_time_ns

    out = np.empty((B, S, S), dtype=np.float32)
    for b in range(B):
        acc = res.results[b * HPC]["oT_t"].astype(np.float32)
        for hg in range(1, HPC):
            acc = acc + res.results[b * HPC + hg]["oT_t"].astype(np.float32)
        # [qc, iblk, p, t] -> [iblk*128+p, qc*512+t] = oT_full, out = oT_full.T
        o_full = acc.transpose(1, 2, 0, 3).reshape(S, S)
        out[b] = o_full.T
    return out
